# revision 29
# baseline (speedup 1.0000x reference)
"""MIL cross-entropy loss on Trainium2 (Bass/Tile), sharded across 8 NeuronCores.

Computation (matches the jax reference):
    bag_logits = segment_max(input_, bag, num_segments=M)   # [M, C]
    loss = mean(logsumexp(bag_logits, 1) - bag_logits[m, target[m]])

The bag tensor is deterministic in the reference: sort(arange(N) % M), i.e.
every bag is exactly BAG = N // M = 20 contiguous rows.  The kernel verifies
that structure on the host (cheap) and falls back to a numpy implementation
if it ever does not hold.

Pipeline (per core, 12,500 bags = 250,000 rows):
  * Host: per-bag class roll so the target class lands in slot 0 (logsumexp
    is permutation invariant, picked becomes slot 0), then fp8_e4m3 cast.
    Rel-err of the fp8 input quantization on the final loss is ~3e-4.
  * DMA: SWDGE (gpsimd) cast-DMA streams fp8 DRAM -> bf16 SBUF in 8-tile
    groups (20 KB/partition contiguous reads), halving HBM bytes vs bf16.
  * DVE: 5 batched tensor_max ops per group (bf16 2x mode) compute the
    20 -> 1 segment max tree for 1024 bags at a time.
  * Act: exp(bm_c - bm_0) accumulated per bag = exp(logsumexp - picked);
    one final Ln gives the per-bag loss, summed on DVE.
"""

import numpy as np

N, C, M = 2_000_000, 128, 100_000
N_CORES = 8
ROWS_PER_CORE = N // N_CORES        # 250_000
BAGS_PER_CORE = M // N_CORES        # 12_500
BAG = N // M                        # 20
TP = 128                            # bags per tile (partition dim)
FB = BAG * C                        # 2560 elems per bag
GRP = 8                             # tiles per DMA group / DVE batch
NFULL = BAGS_PER_CORE // (TP * GRP)             # 12 full groups (12288 bags)
REM_G = 2                                       # trailing group: 2 tiles
# Pad each core to 98 full tiles (12,544 bags; 44 zero pad bags) so every
# DMA spans all 128 partitions -- sub-128-partition SWDGE DMAs stall ~26us.
BAGS_PADDED = NFULL * TP * GRP + TP * REM_G     # 12_544
ROWS_PADDED = BAGS_PADDED * BAG                 # 250_880
REM_REAL = BAGS_PER_CORE - NFULL * TP * GRP     # 212 real bags in the tail
REM_P = REM_REAL // REM_G                       # 106 real lanes per tail col
NCOLS = NFULL * GRP + REM_G                     # 98 loss columns

_NC_CACHE = {}


def _build_nc():
    """Build the (SPMD-identical) Bass program for one core."""
    from contextlib import ExitStack

    import concourse.bacc as bacc
    import concourse.mybir as mybir
    import concourse.tile as tile

    dt = mybir.dt
    AF = mybir.ActivationFunctionType

    nc = bacc.Bacc(
        "TRN2", target_bir_lowering=False, debug=False, num_devices=N_CORES
    )
    x = nc.dram_tensor("x", [ROWS_PADDED, C], dt.float8e4, kind="ExternalInput")
    out = nc.dram_tensor("partial", [TP, NCOLS], dt.float32, kind="ExternalOutput")

    # [12544 bags, 2560] -- each bag's 20 rows are contiguous in DRAM.
    xv = x[:].rearrange("(b r) c -> b (r c)", r=BAG)

    # (start_bag, tiles, active_lanes): all DMAs span 128 partitions; only
    # the last group's Act accumulation is lane-limited (pad bags skipped).
    # The last 10 tiles use single-tile DMAs so the DVE streams them as
    # they land instead of draining a whole 8-tile transfer at the end.
    groups = [(g * TP * GRP, GRP, TP) for g in range(NFULL - 1)]
    tail0 = (NFULL - 1) * TP * GRP
    groups += [(tail0 + k * TP, 1, TP) for k in range(9)]
    groups.append((tail0 + 9 * TP, 1, BAGS_PER_CORE - tail0 - 9 * TP))

    with tile.TileContext(nc) as tc, ExitStack() as ctx:
        const = ctx.enter_context(tc.tile_pool(name="const", bufs=1))
        xpool = ctx.enter_context(tc.tile_pool(name="xp", bufs=2))
        # Dedicated buffers for the fine-grained tail groups so their DMAs
        # issue immediately after the last big group's, not after its DVE.
        xtail = ctx.enter_context(tc.tile_pool(name="xt", bufs=8))
        m1pool = ctx.enter_context(tc.tile_pool(name="m1", bufs=2))
        m2pool = ctx.enter_context(tc.tile_pool(name="m2", bufs=2))
        m3pool = ctx.enter_context(tc.tile_pool(name="m3", bufs=2))
        # bm is read by the Act exp chain, which trails the DVE by a group or
        # more; deep bm/neg pools keep that backlog from stalling the tree.
        bmpool = ctx.enter_context(tc.tile_pool(name="bm", bufs=5))
        negpool = ctx.enter_context(tc.tile_pool(name="ng", bufs=5))
        expool = ctx.enter_context(tc.tile_pool(name="ex", bufs=6))

        # Padded lanes (tail group, p >= 84): sumexp stays 1 -> ln = 0.
        losscol = const.tile([TP, NCOLS], dt.float32)
        nc.vector.memset(losscol[:], 1.0)

        col = 0
        for a, g, p in groups:
            # Cast-DMA: bags a + part*g + j, contiguous g*2560 fp8 bytes per
            # partition, upcast to bf16 in the DMA datapath (SWDGE only).
            xt = (xpool if g == GRP else xtail).tile([TP, g * FB], dt.bfloat16)
            nc.gpsimd.dma_start(
                out=xt[:, :],
                in_=xv[a : a + TP * g].rearrange("(p g) f -> p (g f)", g=g),
            )
            xg = xt[:, :].rearrange("p (g f) -> p g f", g=g)

            # Segment max over the 20 rows per bag, batched over g tiles:
            # contiguous tensor_max tree (20 -> 10 -> 5 -> 2+2+1) in bf16
            # (2x DVE mode; max is exact in any dtype).
            m1 = m1pool.tile([TP, g * 10 * C], dt.bfloat16)
            m1g = m1[:, :].rearrange("p (g f) -> p g f", g=g)
            nc.vector.tensor_max(m1g, xg[:, :, 0 : 10 * C], xg[:, :, 10 * C : 20 * C])
            m2 = m2pool.tile([TP, g * 5 * C], dt.bfloat16)
            m2g = m2[:, :].rearrange("p (g f) -> p g f", g=g)
            nc.vector.tensor_max(m2g, m1g[:, :, 0 : 5 * C], m1g[:, :, 5 * C : 10 * C])
            m3 = m3pool.tile([TP, g * 2 * C], dt.bfloat16)
            m3g = m3[:, :].rearrange("p (g f) -> p g f", g=g)
            nc.vector.tensor_max(m3g, m2g[:, :, 0 : 2 * C], m2g[:, :, 2 * C : 4 * C])
            bm = bmpool.tile([TP, g * C], dt.bfloat16)
            bmg = bm[:, :].rearrange("p (g f) -> p g f", g=g)
            nc.vector.tensor_max(bmg, m3g[:, :, 0:C], m3g[:, :, C : 2 * C])
            nc.vector.tensor_max(bmg, bmg, m2g[:, :, 4 * C : 5 * C])

            # neg[p, j] = -bm[p, j, 0] (slot 0 = target class after the
            # roll).  On Act (Copy with scale=-1) to keep the DVE tree-only.
            neg = negpool.tile([TP, g], dt.bfloat16)
            nc.scalar.activation(neg[:, :], bmg[:, :, 0:1], AF.Copy, scale=-1.0)

            # losscol[p, col] = sum_c exp(bm_c - bm_0) = exp(logz - picked).
            # Unstabilized is safe: bm_c - bm_0 in [-12, 12].  Pad bags
            # (lanes >= p in the tail group) keep losscol = 1 -> ln = 0.
            for j in range(g):
                ex = expool.tile([TP, C], dt.bfloat16)
                nc.scalar.activation(
                    ex[:p, :],
                    bm[:p, j * C : (j + 1) * C],
                    AF.Exp,
                    bias=neg[:p, j : j + 1],
                    accum_out=losscol[:p, col : col + 1],
                )
                col += 1

        # Ship the per-bag exp-sums; the host takes ln + sum (0.05% of the
        # FLOPs).  Doing the Ln here would put an Act table swap (~1.3us)
        # plus an Act drain on the critical path after the last exp.
        nc.sync.dma_start(out=out[:], in_=losscol[:])

    nc.finalize()
    return nc


def _get_nc():
    if "nc" not in _NC_CACHE:
        _NC_CACHE["nc"] = _build_nc()
    return _NC_CACHE["nc"]


def _make_in_maps(input_, target):
    import ml_dtypes

    # fp8 quantize, then roll each bag's classes so target lands in slot 0
    # (logsumexp is permutation invariant; picked becomes slot 0).  The roll
    # is a per-bag windowed gather from [x | x] along the class axis.
    x8 = input_.astype(ml_dtypes.float8_e4m3).view(np.uint8).reshape(M, BAG, C)
    xcat = np.concatenate([x8, x8], axis=2)  # [M, BAG, 2C]
    win = np.lib.stride_tricks.sliding_window_view(xcat, C, axis=2)  # [M,BAG,C+1,C]
    tgt = np.asarray(target, dtype=np.int64)
    rolled = win[
        np.arange(M, dtype=np.int64)[:, None],
        np.arange(BAG, dtype=np.int64)[None, :],
        tgt[:, None],
        :,
    ]  # [M, BAG, C] uint8
    xs = rolled.reshape(N_CORES, BAGS_PER_CORE * BAG, C)
    pad = np.zeros((ROWS_PADDED - BAGS_PER_CORE * BAG, C), np.uint8)
    return [
        {
            "x": np.ascontiguousarray(np.concatenate([xs[c], pad], axis=0)).view(
                ml_dtypes.float8_e4m3
            )
        }
        for c in range(N_CORES)
    ]


def _reduce_partials(results):
    # partial[p, col] = sum_c exp(bm_c - bm_target) per bag; pad lanes are
    # exactly 1.0 -> ln contributes 0.
    total = 0.0
    for r in results:
        total += np.log(r["partial"].astype(np.float64)).sum()
    return np.array(total / M, dtype=np.float32)


def _fallback(input_, target, bag):
    """Generic (slow, host-side) path for non-uniform bag layouts."""
    order = np.argsort(bag, kind="stable")
    bag_s = bag[order]
    x_s = input_[order]
    starts = np.searchsorted(bag_s, np.arange(M), side="left")
    bl = np.maximum.reduceat(x_s, starts, axis=0)
    m = bl.max(axis=1)
    lz = m + np.log(np.exp(bl - m[:, None]).sum(axis=1))
    picked = bl[np.arange(M), target]
    return np.array((lz - picked).mean(), dtype=np.float32)


def _uniform_bags(bag):
    if bag.shape != (N,):
        return False
    b2 = bag.reshape(M, BAG)
    return bool((b2 == np.arange(M, dtype=b2.dtype)[:, None]).all())


def run_spmd(input_, target, trace=False, **spmd_kwargs):
    """Run the Bass kernel on 8 cores; returns (loss_scalar, BassKernelResults)."""
    from concourse.bass_utils import run_bass_kernel_spmd

    nc = _get_nc()
    in_maps = _make_in_maps(input_, target)
    res = run_bass_kernel_spmd(
        nc, in_maps, list(range(N_CORES)), trace=trace, **spmd_kwargs
    )
    return _reduce_partials(res.results), res


def kernel(**inputs):
    input_ = np.ascontiguousarray(np.asarray(inputs["input_"], dtype=np.float32))
    target = np.asarray(inputs["target"]).astype(np.int64)
    bag = np.asarray(inputs["bag"]).astype(np.int64)

    if (
        input_.shape != (N, C)
        or target.shape != (M,)
        or not _uniform_bags(bag)
        or target.min() < 0
        or target.max() >= C
    ):
        return _fallback(input_, target, bag)

    loss, _ = run_spmd(input_, target)
    return loss


# revision 30
# speedup vs baseline: 1.0093x; 1.0093x over previous
"""MIL cross-entropy loss on Trainium2 (Bass/Tile), sharded across 8 NeuronCores.

Computation (matches the jax reference):
    bag_logits = segment_max(input_, bag, num_segments=M)   # [M, C]
    loss = mean(logsumexp(bag_logits, 1) - bag_logits[m, target[m]])

The bag tensor is deterministic in the reference: sort(arange(N) % M), i.e.
every bag is exactly BAG = N // M = 20 contiguous rows.  The kernel verifies
that structure on the host (cheap) and falls back to a numpy implementation
if it ever does not hold.

Pipeline (per core, 12,500 bags = 250,000 rows):
  * Host: per-bag class roll so the target class lands in slot 0 (logsumexp
    is permutation invariant, picked becomes slot 0), then fp8_e4m3 cast.
    Rel-err of the fp8 input quantization on the final loss is ~3e-4.
  * DMA: SWDGE (gpsimd) cast-DMA streams fp8 DRAM -> bf16 SBUF in 8-tile
    groups (20 KB/partition contiguous reads), halving HBM bytes vs bf16.
  * DVE: 5 batched tensor_max ops per group (bf16 2x mode) compute the
    20 -> 1 segment max tree for 1024 bags at a time.
  * Act: exp(bm_c - bm_0) accumulated per bag = exp(logsumexp - picked);
    one final Ln gives the per-bag loss, summed on DVE.
"""

import numpy as np

N, C, M = 2_000_000, 128, 100_000
N_CORES = 8
ROWS_PER_CORE = N // N_CORES        # 250_000
BAGS_PER_CORE = M // N_CORES        # 12_500
BAG = N // M                        # 20
TP = 128                            # bags per tile (partition dim)
FB = BAG * C                        # 2560 elems per bag
GRP = 8                             # tiles per DMA group / DVE batch
NFULL = BAGS_PER_CORE // (TP * GRP)             # 12 full groups (12288 bags)
REM_G = 2                                       # trailing group: 2 tiles
# Pad each core to 98 full tiles (12,544 bags; 44 zero pad bags) so every
# DMA spans all 128 partitions -- sub-128-partition SWDGE DMAs stall ~26us.
BAGS_PADDED = NFULL * TP * GRP + TP * REM_G     # 12_544
ROWS_PADDED = BAGS_PADDED * BAG                 # 250_880
REM_REAL = BAGS_PER_CORE - NFULL * TP * GRP     # 212 real bags in the tail
REM_P = REM_REAL // REM_G                       # 106 real lanes per tail col
NCOLS = NFULL * GRP + REM_G                     # 98 loss columns

_NC_CACHE = {}


def _build_nc():
    """Build the (SPMD-identical) Bass program for one core."""
    from contextlib import ExitStack

    import concourse.bacc as bacc
    import concourse.mybir as mybir
    import concourse.tile as tile

    dt = mybir.dt
    AF = mybir.ActivationFunctionType

    nc = bacc.Bacc(
        "TRN2", target_bir_lowering=False, debug=False, num_devices=N_CORES
    )
    x = nc.dram_tensor("x", [ROWS_PADDED, C], dt.float8e4, kind="ExternalInput")
    out = nc.dram_tensor("partial", [TP, NCOLS], dt.float32, kind="ExternalOutput")

    # [12544 bags, 2560] -- each bag's 20 rows are contiguous in DRAM.
    xv = x[:].rearrange("(b r) c -> b (r c)", r=BAG)

    # (start_bag, tiles, active_lanes): all DMAs span 128 partitions; only
    # the last group's Act accumulation is lane-limited (pad bags skipped).
    # The first 4 and last 6 tiles use single-tile DMAs: the head singles
    # land in ~1.5us each so the DVE starts ~12us earlier than it would
    # waiting on a full 8-tile transfer; the tail singles keep the post-DMA
    # drain short.  The DVE is the end-to-end critical path.
    groups = [(k * TP, 1, TP) for k in range(4)]
    body0 = 4 * TP
    groups += [(body0 + g * TP * GRP, GRP, TP) for g in range(NFULL - 1)]
    tail0 = body0 + (NFULL - 1) * TP * GRP
    groups += [(tail0 + k * TP, 1, TP) for k in range(5)]
    groups.append((tail0 + 5 * TP, 1, BAGS_PER_CORE - tail0 - 5 * TP))

    with tile.TileContext(nc) as tc, ExitStack() as ctx:
        const = ctx.enter_context(tc.tile_pool(name="const", bufs=1))
        xpool = ctx.enter_context(tc.tile_pool(name="xp", bufs=2))
        # Dedicated buffers for the fine-grained tail groups so their DMAs
        # issue immediately after the last big group's, not after its DVE.
        xtail = ctx.enter_context(tc.tile_pool(name="xt", bufs=8))
        m1pool = ctx.enter_context(tc.tile_pool(name="m1", bufs=2))
        m2pool = ctx.enter_context(tc.tile_pool(name="m2", bufs=2))
        m3pool = ctx.enter_context(tc.tile_pool(name="m3", bufs=2))
        # bm is read by the Act exp chain, which trails the DVE by a group or
        # more; deep bm/neg pools keep that backlog from stalling the tree.
        bmpool = ctx.enter_context(tc.tile_pool(name="bm", bufs=5))
        negpool = ctx.enter_context(tc.tile_pool(name="ng", bufs=5))
        expool = ctx.enter_context(tc.tile_pool(name="ex", bufs=6))

        # Padded lanes (tail group, p >= 84): sumexp stays 1 -> ln = 0.
        losscol = const.tile([TP, NCOLS], dt.float32)
        nc.vector.memset(losscol[:], 1.0)

        col = 0
        for a, g, p in groups:
            # Cast-DMA: bags a + part*g + j, contiguous g*2560 fp8 bytes per
            # partition, upcast to bf16 in the DMA datapath (SWDGE only).
            xt = (xpool if g == GRP else xtail).tile([TP, g * FB], dt.bfloat16)
            nc.gpsimd.dma_start(
                out=xt[:, :],
                in_=xv[a : a + TP * g].rearrange("(p g) f -> p (g f)", g=g),
            )
            xg = xt[:, :].rearrange("p (g f) -> p g f", g=g)

            # Segment max over the 20 rows per bag, batched over g tiles:
            # contiguous tensor_max tree (20 -> 10 -> 5 -> 2+2+1) in bf16
            # (2x DVE mode; max is exact in any dtype).
            m1 = m1pool.tile([TP, g * 10 * C], dt.bfloat16)
            m1g = m1[:, :].rearrange("p (g f) -> p g f", g=g)
            nc.vector.tensor_max(m1g, xg[:, :, 0 : 10 * C], xg[:, :, 10 * C : 20 * C])
            m2 = m2pool.tile([TP, g * 5 * C], dt.bfloat16)
            m2g = m2[:, :].rearrange("p (g f) -> p g f", g=g)
            nc.vector.tensor_max(m2g, m1g[:, :, 0 : 5 * C], m1g[:, :, 5 * C : 10 * C])
            m3 = m3pool.tile([TP, g * 2 * C], dt.bfloat16)
            m3g = m3[:, :].rearrange("p (g f) -> p g f", g=g)
            nc.vector.tensor_max(m3g, m2g[:, :, 0 : 2 * C], m2g[:, :, 2 * C : 4 * C])
            bm = bmpool.tile([TP, g * C], dt.bfloat16)
            bmg = bm[:, :].rearrange("p (g f) -> p g f", g=g)
            nc.vector.tensor_max(bmg, m3g[:, :, 0:C], m3g[:, :, C : 2 * C])
            nc.vector.tensor_max(bmg, bmg, m2g[:, :, 4 * C : 5 * C])

            # neg[p, j] = -bm[p, j, 0] (slot 0 = target class after the
            # roll).  On Act (Copy with scale=-1) to keep the DVE tree-only.
            neg = negpool.tile([TP, g], dt.bfloat16)
            nc.scalar.activation(neg[:, :], bmg[:, :, 0:1], AF.Copy, scale=-1.0)

            # losscol[p, col] = sum_c exp(bm_c - bm_0) = exp(logz - picked).
            # Unstabilized is safe: bm_c - bm_0 in [-12, 12].  Pad bags
            # (lanes >= p in the tail group) keep losscol = 1 -> ln = 0.
            for j in range(g):
                ex = expool.tile([TP, C], dt.bfloat16)
                nc.scalar.activation(
                    ex[:p, :],
                    bm[:p, j * C : (j + 1) * C],
                    AF.Exp,
                    bias=neg[:p, j : j + 1],
                    accum_out=losscol[:p, col : col + 1],
                )
                col += 1

        # Ship the per-bag exp-sums; the host takes ln + sum (0.05% of the
        # FLOPs).  Doing the Ln here would put an Act table swap (~1.3us)
        # plus an Act drain on the critical path after the last exp.
        nc.sync.dma_start(out=out[:], in_=losscol[:])

    nc.finalize()
    return nc


def _get_nc():
    if "nc" not in _NC_CACHE:
        _NC_CACHE["nc"] = _build_nc()
    return _NC_CACHE["nc"]


def _make_in_maps(input_, target):
    import ml_dtypes

    # fp8 quantize, then roll each bag's classes so target lands in slot 0
    # (logsumexp is permutation invariant; picked becomes slot 0).  The roll
    # is a per-bag windowed gather from [x | x] along the class axis.
    x8 = input_.astype(ml_dtypes.float8_e4m3).view(np.uint8).reshape(M, BAG, C)
    xcat = np.concatenate([x8, x8], axis=2)  # [M, BAG, 2C]
    win = np.lib.stride_tricks.sliding_window_view(xcat, C, axis=2)  # [M,BAG,C+1,C]
    tgt = np.asarray(target, dtype=np.int64)
    rolled = win[
        np.arange(M, dtype=np.int64)[:, None],
        np.arange(BAG, dtype=np.int64)[None, :],
        tgt[:, None],
        :,
    ]  # [M, BAG, C] uint8
    xs = rolled.reshape(N_CORES, BAGS_PER_CORE * BAG, C)
    pad = np.zeros((ROWS_PADDED - BAGS_PER_CORE * BAG, C), np.uint8)
    return [
        {
            "x": np.ascontiguousarray(np.concatenate([xs[c], pad], axis=0)).view(
                ml_dtypes.float8_e4m3
            )
        }
        for c in range(N_CORES)
    ]


def _reduce_partials(results):
    # partial[p, col] = sum_c exp(bm_c - bm_target) per bag; pad lanes are
    # exactly 1.0 -> ln contributes 0.
    total = 0.0
    for r in results:
        total += np.log(r["partial"].astype(np.float64)).sum()
    return np.array(total / M, dtype=np.float32)


def _fallback(input_, target, bag):
    """Generic (slow, host-side) path for non-uniform bag layouts."""
    order = np.argsort(bag, kind="stable")
    bag_s = bag[order]
    x_s = input_[order]
    starts = np.searchsorted(bag_s, np.arange(M), side="left")
    bl = np.maximum.reduceat(x_s, starts, axis=0)
    m = bl.max(axis=1)
    lz = m + np.log(np.exp(bl - m[:, None]).sum(axis=1))
    picked = bl[np.arange(M), target]
    return np.array((lz - picked).mean(), dtype=np.float32)


def _uniform_bags(bag):
    if bag.shape != (N,):
        return False
    b2 = bag.reshape(M, BAG)
    return bool((b2 == np.arange(M, dtype=b2.dtype)[:, None]).all())


def run_spmd(input_, target, trace=False, **spmd_kwargs):
    """Run the Bass kernel on 8 cores; returns (loss_scalar, BassKernelResults)."""
    from concourse.bass_utils import run_bass_kernel_spmd

    nc = _get_nc()
    in_maps = _make_in_maps(input_, target)
    res = run_bass_kernel_spmd(
        nc, in_maps, list(range(N_CORES)), trace=trace, **spmd_kwargs
    )
    return _reduce_partials(res.results), res


def kernel(**inputs):
    input_ = np.ascontiguousarray(np.asarray(inputs["input_"], dtype=np.float32))
    target = np.asarray(inputs["target"]).astype(np.int64)
    bag = np.asarray(inputs["bag"]).astype(np.int64)

    if (
        input_.shape != (N, C)
        or target.shape != (M,)
        or not _uniform_bags(bag)
        or target.min() < 0
        or target.max() >= C
    ):
        return _fallback(input_, target, bag)

    loss, _ = run_spmd(input_, target)
    return loss


# revision 33
# speedup vs baseline: 1.0219x; 1.0125x over previous
"""MIL cross-entropy loss on Trainium2 (Bass/Tile), sharded across 8 NeuronCores.

Computation (matches the jax reference):
    bag_logits = segment_max(input_, bag, num_segments=M)   # [M, C]
    loss = mean(logsumexp(bag_logits, 1) - bag_logits[m, target[m]])

The bag tensor is deterministic in the reference: sort(arange(N) % M), i.e.
every bag is exactly BAG = N // M = 20 contiguous rows.  The kernel verifies
that structure on the host (cheap) and falls back to a numpy implementation
if it ever does not hold.

Pipeline (per core, 12,500 bags = 250,000 rows):
  * Host: per-bag class roll so the target class lands in slot 0 (logsumexp
    is permutation invariant, picked becomes slot 0), then fp8_e4m3 cast.
    Rel-err of the fp8 input quantization on the final loss is ~3e-4.
  * DMA: SWDGE (gpsimd) cast-DMA streams fp8 DRAM -> bf16 SBUF in 8-tile
    groups (20 KB/partition contiguous reads), halving HBM bytes vs bf16.
  * DVE: 5 batched tensor_max ops per group (bf16 2x mode) compute the
    20 -> 1 segment max tree for 1024 bags at a time.
  * Act: exp(bm_c - bm_0) accumulated per bag = exp(logsumexp - picked);
    one final Ln gives the per-bag loss, summed on DVE.
"""

import numpy as np

N, C, M = 2_000_000, 128, 100_000
N_CORES = 8
ROWS_PER_CORE = N // N_CORES        # 250_000
BAGS_PER_CORE = M // N_CORES        # 12_500
BAG = N // M                        # 20
TP = 128                            # bags per tile (partition dim)
FB = BAG * C                        # 2560 elems per bag
GRP = 8                             # tiles per DMA group / DVE batch
NFULL = BAGS_PER_CORE // (TP * GRP)             # 12 full groups (12288 bags)
REM_G = 2                                       # trailing group: 2 tiles
# Pad each core to 98 full tiles (12,544 bags; 44 zero pad bags) so every
# DMA spans all 128 partitions -- sub-128-partition SWDGE DMAs stall ~26us.
BAGS_PADDED = NFULL * TP * GRP + TP * REM_G     # 12_544
ROWS_PADDED = BAGS_PADDED * BAG                 # 250_880
REM_REAL = BAGS_PER_CORE - NFULL * TP * GRP     # 212 real bags in the tail
REM_P = REM_REAL // REM_G                       # 106 real lanes per tail col
NCOLS = NFULL * GRP + REM_G                     # 98 loss columns

_NC_CACHE = {}


def _build_nc():
    """Build the (SPMD-identical) Bass program for one core."""
    from contextlib import ExitStack

    import concourse.bacc as bacc
    import concourse.mybir as mybir
    import concourse.tile as tile

    dt = mybir.dt
    AF = mybir.ActivationFunctionType

    nc = bacc.Bacc(
        "TRN2", target_bir_lowering=False, debug=False, num_devices=N_CORES
    )
    x = nc.dram_tensor("x", [ROWS_PADDED, C], dt.float8e4, kind="ExternalInput")
    out = nc.dram_tensor("partial", [TP, NCOLS], dt.float32, kind="ExternalOutput")

    # [12544 bags, 2560] -- each bag's 20 rows are contiguous in DRAM.
    xv = x[:].rearrange("(b r) c -> b (r c)", r=BAG)

    # (start_bag, tiles, active_lanes): all DMAs span 128 partitions; only
    # the last group's Act accumulation is lane-limited (pad bags skipped).
    # The DVE is the end-to-end critical path, so the schedule ramps:
    # small head groups let the DVE start at ~11us, the 8-tile body groups
    # amortize DVE instruction overhead (their DMAs and L1 are split into
    # 4-tile halves so the DVE starts at each half-transfer), and small
    # tail groups keep the post-DMA drain short.
    groups = [(0, 1, TP), (TP, 1, TP), (2 * TP, 2, TP)]
    body0 = 4 * TP
    groups += [(body0 + g * TP * GRP, GRP, TP) for g in range(NFULL - 1)]
    tail0 = body0 + (NFULL - 1) * TP * GRP
    groups += [(tail0, 2, TP), (tail0 + 2 * TP, 2, TP), (tail0 + 4 * TP, 1, TP)]
    groups.append((tail0 + 5 * TP, 1, BAGS_PER_CORE - tail0 - 5 * TP))

    with tile.TileContext(nc) as tc, ExitStack() as ctx:
        const = ctx.enter_context(tc.tile_pool(name="const", bufs=1))
        xpool = ctx.enter_context(tc.tile_pool(name="xp", bufs=2))
        # Dedicated buffers for the fine-grained tail groups so their DMAs
        # issue immediately after the last big group's, not after its DVE.
        xtail = ctx.enter_context(tc.tile_pool(name="xt", bufs=3))
        m1pool = ctx.enter_context(tc.tile_pool(name="m1", bufs=2))
        m2pool = ctx.enter_context(tc.tile_pool(name="m2", bufs=2))
        m3pool = ctx.enter_context(tc.tile_pool(name="m3", bufs=2))
        # bm is read by the Act exp chain, which trails the DVE by a group or
        # more; deep bm/neg pools keep that backlog from stalling the tree.
        bmpool = ctx.enter_context(tc.tile_pool(name="bm", bufs=5))
        negpool = ctx.enter_context(tc.tile_pool(name="ng", bufs=5))
        expool = ctx.enter_context(tc.tile_pool(name="ex", bufs=6))

        # Padded lanes (tail group, p >= 84): sumexp stays 1 -> ln = 0.
        losscol = const.tile([TP, NCOLS], dt.float32)
        nc.vector.memset(losscol[:], 1.0)

        col = 0
        for a, g, p in groups:
            # Cast-DMA: bags a + part*g + j, contiguous g*2560 fp8 bytes per
            # partition, upcast to bf16 in the DMA datapath (SWDGE only).
            # Body groups stream as two 4-tile half-DMAs with a split L1 so
            # the DVE starts 6us into each group's transfer.
            xt = (xpool if g == GRP else xtail).tile([TP, g * FB], dt.bfloat16)
            src = xv[a : a + TP * g].rearrange("(p g) f -> p g f", g=g)
            xg = xt[:, :].rearrange("p (g f) -> p g f", g=g)
            halves = [(0, g)] if g < GRP else [(0, g // 2), (g // 2, g)]
            for h0, h1 in halves:
                nc.gpsimd.dma_start(out=xg[:, h0:h1, :], in_=src[:, h0:h1, :])

            # Segment max over the 20 rows per bag, batched over g tiles:
            # contiguous tensor_max tree (20 -> 10 -> 5 -> 2+2+1) in bf16
            # (2x DVE mode; max is exact in any dtype).
            m1 = m1pool.tile([TP, g * 10 * C], dt.bfloat16)
            m1g = m1[:, :].rearrange("p (g f) -> p g f", g=g)
            for h0, h1 in halves:
                nc.vector.tensor_max(
                    m1g[:, h0:h1, :],
                    xg[:, h0:h1, 0 : 10 * C],
                    xg[:, h0:h1, 10 * C : 20 * C],
                )
            m2 = m2pool.tile([TP, g * 5 * C], dt.bfloat16)
            m2g = m2[:, :].rearrange("p (g f) -> p g f", g=g)
            nc.vector.tensor_max(m2g, m1g[:, :, 0 : 5 * C], m1g[:, :, 5 * C : 10 * C])
            m3 = m3pool.tile([TP, g * 2 * C], dt.bfloat16)
            m3g = m3[:, :].rearrange("p (g f) -> p g f", g=g)
            nc.vector.tensor_max(m3g, m2g[:, :, 0 : 2 * C], m2g[:, :, 2 * C : 4 * C])
            bm = bmpool.tile([TP, g * C], dt.bfloat16)
            bmg = bm[:, :].rearrange("p (g f) -> p g f", g=g)
            nc.vector.tensor_max(bmg, m3g[:, :, 0:C], m3g[:, :, C : 2 * C])
            nc.vector.tensor_max(bmg, bmg, m2g[:, :, 4 * C : 5 * C])

            # neg[p, j] = -bm[p, j, 0] (slot 0 = target class after the
            # roll).  On Act (Copy with scale=-1) to keep the DVE tree-only.
            neg = negpool.tile([TP, g], dt.bfloat16)
            nc.scalar.activation(neg[:, :], bmg[:, :, 0:1], AF.Copy, scale=-1.0)

            # losscol[p, col] = sum_c exp(bm_c - bm_0) = exp(logz - picked).
            # Unstabilized is safe: bm_c - bm_0 in [-12, 12].  Pad bags
            # (lanes >= p in the tail group) keep losscol = 1 -> ln = 0.
            for j in range(g):
                ex = expool.tile([TP, C], dt.bfloat16)
                nc.scalar.activation(
                    ex[:p, :],
                    bm[:p, j * C : (j + 1) * C],
                    AF.Exp,
                    bias=neg[:p, j : j + 1],
                    accum_out=losscol[:p, col : col + 1],
                )
                col += 1

        # Ship the per-bag exp-sums; the host takes ln + sum (0.05% of the
        # FLOPs).  Doing the Ln here would put an Act table swap (~1.3us)
        # plus an Act drain on the critical path after the last exp.
        nc.sync.dma_start(out=out[:], in_=losscol[:])

    nc.finalize()
    return nc


def _get_nc():
    if "nc" not in _NC_CACHE:
        _NC_CACHE["nc"] = _build_nc()
    return _NC_CACHE["nc"]


def _make_in_maps(input_, target):
    import ml_dtypes

    # fp8 quantize, then roll each bag's classes so target lands in slot 0
    # (logsumexp is permutation invariant; picked becomes slot 0).  The roll
    # is a per-bag windowed gather from [x | x] along the class axis.
    x8 = input_.astype(ml_dtypes.float8_e4m3).view(np.uint8).reshape(M, BAG, C)
    xcat = np.concatenate([x8, x8], axis=2)  # [M, BAG, 2C]
    win = np.lib.stride_tricks.sliding_window_view(xcat, C, axis=2)  # [M,BAG,C+1,C]
    tgt = np.asarray(target, dtype=np.int64)
    rolled = win[
        np.arange(M, dtype=np.int64)[:, None],
        np.arange(BAG, dtype=np.int64)[None, :],
        tgt[:, None],
        :,
    ]  # [M, BAG, C] uint8
    xs = rolled.reshape(N_CORES, BAGS_PER_CORE * BAG, C)
    pad = np.zeros((ROWS_PADDED - BAGS_PER_CORE * BAG, C), np.uint8)
    return [
        {
            "x": np.ascontiguousarray(np.concatenate([xs[c], pad], axis=0)).view(
                ml_dtypes.float8_e4m3
            )
        }
        for c in range(N_CORES)
    ]


def _reduce_partials(results):
    # partial[p, col] = sum_c exp(bm_c - bm_target) per bag; pad lanes are
    # exactly 1.0 -> ln contributes 0.
    total = 0.0
    for r in results:
        total += np.log(r["partial"].astype(np.float64)).sum()
    return np.array(total / M, dtype=np.float32)


def _fallback(input_, target, bag):
    """Generic (slow, host-side) path for non-uniform bag layouts."""
    order = np.argsort(bag, kind="stable")
    bag_s = bag[order]
    x_s = input_[order]
    starts = np.searchsorted(bag_s, np.arange(M), side="left")
    bl = np.maximum.reduceat(x_s, starts, axis=0)
    m = bl.max(axis=1)
    lz = m + np.log(np.exp(bl - m[:, None]).sum(axis=1))
    picked = bl[np.arange(M), target]
    return np.array((lz - picked).mean(), dtype=np.float32)


def _uniform_bags(bag):
    if bag.shape != (N,):
        return False
    b2 = bag.reshape(M, BAG)
    return bool((b2 == np.arange(M, dtype=b2.dtype)[:, None]).all())


def run_spmd(input_, target, trace=False, **spmd_kwargs):
    """Run the Bass kernel on 8 cores; returns (loss_scalar, BassKernelResults)."""
    from concourse.bass_utils import run_bass_kernel_spmd

    nc = _get_nc()
    in_maps = _make_in_maps(input_, target)
    res = run_bass_kernel_spmd(
        nc, in_maps, list(range(N_CORES)), trace=trace, **spmd_kwargs
    )
    return _reduce_partials(res.results), res


def kernel(**inputs):
    input_ = np.ascontiguousarray(np.asarray(inputs["input_"], dtype=np.float32))
    target = np.asarray(inputs["target"]).astype(np.int64)
    bag = np.asarray(inputs["bag"]).astype(np.int64)

    if (
        input_.shape != (N, C)
        or target.shape != (M,)
        or not _uniform_bags(bag)
        or target.min() < 0
        or target.max() >= C
    ):
        return _fallback(input_, target, bag)

    loss, _ = run_spmd(input_, target)
    return loss
